# revision 5
# baseline (speedup 1.0000x reference)
"""GNN scatter-mean + Linear kernel for Trainium2, 8 NeuronCores.

Strategy (node-sharded, fp8 DoubleRow, no collectives):
  - CPU: sort edges by destination node, bucket per core (each core owns 1250
    contiguous nodes = 10 tiles of 128). Edge features are shipped RAW
    (unscaled) quantized to fp8 e4m3 with error-feedback (noise-shaped)
    rounding along each node's edge chain: the running quantization error is
    carried into the next edge of the same node, so the device-side segment
    sum sees only ~one ulp of error instead of sqrt(count) ulps. The 1/count
    mean division (and the fp8 range scale) is applied on-device per node
    after aggregation.
  - Slots hold 256 edges as [128 partitions, 2 packed] consumed by a single
    DoubleRow fp8 matmul (2 contractions of K=128 per instruction, 128 PE
    cycles per slot): identity slots (partition p, both halves -> node p) use
    one constant [128,2,128] fp8 weight tile loaded once (redundant Ldweights
    stripped post-compile); overflow edges use per-slot one-hot weights built
    on DVE (iota + is_equal, one op per packed half). PSUM accumulates fp32.
  - The whole edge stream (~11 MB/core) fits in SBUF, so every chunk DMA is
    triggered upfront with no pool recycling: DMA streams flat-out at the
    ~435 GB/s fabric limit and the PE (which outpaces DMA even at the cold
    1.2 GHz HAM clock) just follows the stream front.
  - Per node tile: evict PSUM via tensor_scalar multiply with the per-node
    1/(count*SCALE) vector (fp16 out), transpose via PE, apply the 256x256
    Linear (2 K-chunk fp16 matmuls), add bias, DMA out.
  - CPU: concatenate the 8 per-core [1250, 256] blocks.
"""

import sys

sys.path.insert(0, "/opt/trn_rl_repo")

from contextlib import ExitStack

import ml_dtypes
import numpy as np

N_NODES = 10000
N_EDGES = 320000
FEAT = 256
NCORES = 8
P = 128
NPC = (N_NODES + NCORES - 1) // NCORES  # 1250 nodes per core
NTILES = (NPC + P - 1) // P  # 10 node tiles per core
CH = 32  # slots per steady-state DMA chunk (32 * 128 * 2 * 256 * 1B = 2 MiB)
SCALE = 32.0  # fp8 range scale; folded into the on-device recip multiply
W_OVF = 1.3  # cost weight of an overflow slot vs an identity slot

FP8 = ml_dtypes.float8_e4m3fn


def _plan(dst):
    """Choose the shared program structure from the destination indices.

    Slots carry 256 edges ([128 partitions, 2 packed]). caps[t] identity
    slots cover up to 2*caps[t] edges per node; k_ovf[t] one-hot slots take
    the spill. Shared across all 8 cores so one SPMD program serves every
    core.
    """
    perm = np.argsort(dst, kind="stable")
    dst_sorted = dst[perm]
    counts = np.bincount(dst, minlength=N_NODES)

    tile_ranges = []
    for c in range(NCORES):
        rng = []
        for t in range(NTILES):
            n0 = c * NPC + t * P
            n1 = min(c * NPC + min((t + 1) * P, NPC), N_NODES)
            lo = int(np.searchsorted(dst_sorted, n0, side="left"))
            hi = int(np.searchsorted(dst_sorted, n1, side="left"))
            rng.append((lo, hi, n0, n1))
        tile_ranges.append(rng)

    caps, k_ovf = [], []
    for t in range(NTILES):
        cnts = [counts[rng[t][2] : rng[t][3]] for rng in tile_ranges]
        best = None
        for C in range(1, 129):
            ovf_slots = max(
                -(-int(np.maximum(cc - 2 * C, 0).sum()) // (2 * P)) if cc.size else 0
                for cc in cnts
            )
            cost = C + W_OVF * ovf_slots
            if best is None or cost < best[0]:
                best = (cost, C, ovf_slots)
        _, C, ovf_slots = best
        caps.append(C)
        k_ovf.append(ovf_slots)

    base = [0] * (NTILES + 1)
    cur = 0
    for t in range(NTILES):
        base[t] = cur
        cur += caps[t] + k_ovf[t]
    base[NTILES] = cur
    return perm, dst_sorted, counts, tile_ranges, caps, k_ovf, base, cur


def _chunk_schedule(nslot):
    """All chunks are triggered upfront; small head chunks let the PE start
    within ~1us, a small tail chunk keeps the last tile's wait short."""
    head = [2, 2, 4]
    tail = [4, 2]
    sizes = []
    rem = nslot - sum(tail)
    for sz in head:
        if rem <= 0:
            break
        take = min(sz, rem)
        sizes.append(take)
        rem -= take
    while rem > 0:
        take = min(CH, rem)
        sizes.append(take)
        rem -= take
    for sz in tail:
        sizes.append(sz)
    return sizes


def _slot_to_chunk(chunk_sizes):
    m = []
    for ci, sz in enumerate(chunk_sizes):
        for cl in range(sz):
            m.append((ci, cl))
    return m


def _quantize_ef(src, perm, dst_sorted, counts):
    """fp8 e4m3 quantization with per-(node,feature) error feedback.

    Edges are processed in sorted order; the rounding error of edge r of a
    node is added to edge r+1 before rounding, telescoping the segment-sum
    error down to the final edge's single rounding error. Vectorized across
    nodes by rank. Returns codes aligned with the SORTED edge order, plus a
    trailing all-zeros pad row (gather index N_EDGES)."""
    x = src[perm].astype(np.float32) * SCALE
    starts = np.searchsorted(dst_sorted, np.arange(N_NODES)).astype(np.int64)
    q = np.empty((N_EDGES + 1, FEAT), dtype=FP8)
    q[N_EDGES] = 0.0
    carry = np.zeros((N_NODES, FEAT), dtype=np.float32)
    maxc = int(counts.max())
    nodes_all = np.arange(N_NODES)
    for r in range(maxc):
        sel = nodes_all[counts > r]
        eidx = starts[sel] + r
        t = x[eidx] + carry[sel]
        np.clip(t, -239.0, 239.0, out=t)
        qv = t.astype(FP8)
        q[eidx] = qv
        carry[sel] = t - qv.astype(np.float32)
    return q


def _build_program(caps, k_ovf, base, chunk_sizes, nslot, dedup=True):
    from concourse import bacc, mybir
    import concourse.tile as tile

    f32 = mybir.dt.float32
    f16 = mybir.dt.float16
    f8 = mybir.dt.float8e4
    eq = mybir.AluOpType.is_equal
    add = mybir.AluOpType.add
    mult = mybir.AluOpType.mult
    DR = mybir.MatmulPerfMode.DoubleRow

    nc = bacc.Bacc("TRN2", target_bir_lowering=False, debug=False)

    src_drams = [
        nc.dram_tensor(f"src{i}", [P, ch, 2, FEAT], f8, kind="ExternalInput")
        for i, ch in enumerate(chunk_sizes)
    ]
    dstrel_d = nc.dram_tensor("dstrel", [P, 2 * nslot], f32, kind="ExternalInput")
    wt_d = nc.dram_tensor("wt", [P, 2, FEAT], f16, kind="ExternalInput")
    bias_d = nc.dram_tensor("bias", [P, FEAT], f16, kind="ExternalInput")
    iota_d = nc.dram_tensor("iota", [P, P], f16, kind="ExternalInput")
    identw_d = nc.dram_tensor("identw", [P, 2, P], f8, kind="ExternalInput")
    identt_d = nc.dram_tensor("identt", [P, P], f16, kind="ExternalInput")
    recip_d = nc.dram_tensor("recip", [P, NTILES], f32, kind="ExternalInput")
    out_d = nc.dram_tensor("out", [NTILES, P, FEAT], f16, kind="ExternalOutput")

    with tile.TileContext(nc) as tc, ExitStack() as ctx:
        const = ctx.enter_context(tc.tile_pool(name="const", bufs=1))
        srcp = ctx.enter_context(
            tc.tile_pool(name="srcp", bufs=len(chunk_sizes))
        )
        ohp = ctx.enter_context(tc.tile_pool(name="ohp", bufs=8))
        meanp = ctx.enter_context(tc.tile_pool(name="meanp", bufs=2))
        mtp = ctx.enter_context(tc.tile_pool(name="mtp", bufs=2))
        outp = ctx.enter_context(tc.tile_pool(name="outp", bufs=2))
        ps_agg = ctx.enter_context(tc.tile_pool(name="ps_agg", bufs=2, space="PSUM"))
        ps_t = ctx.enter_context(tc.tile_pool(name="ps_t", bufs=2, space="PSUM"))
        ps_out = ctx.enter_context(tc.tile_pool(name="ps_out", bufs=2, space="PSUM"))

        identw_sb = const.tile([P, 2, P], f8)
        nc.scalar.dma_start(identw_sb[:], identw_d[:])
        dstrel_sb = const.tile([P, 2 * nslot], f32)
        nc.scalar.dma_start(dstrel_sb[:], dstrel_d[:])
        iota_sb = const.tile([P, P], f16)
        nc.scalar.dma_start(iota_sb[:], iota_d[:])
        identt_sb = const.tile([P, P], f16)
        nc.scalar.dma_start(identt_sb[:], identt_d[:])
        wt_sb = const.tile([P, 2, FEAT], f16)
        nc.scalar.dma_start(wt_sb[:], wt_d[:])
        bias_sb = const.tile([P, FEAT], f16)
        nc.scalar.dma_start(bias_sb[:], bias_d[:])
        recip_sb = const.tile([P, NTILES], f32)
        nc.scalar.dma_start(recip_sb[:], recip_d[:])

        # the whole edge stream lives in SBUF: trigger every chunk upfront so
        # the DMA engines stream back-to-back with no flow-control coupling
        chunk_tiles = []
        for i, ch in enumerate(chunk_sizes):
            ct = srcp.tile([P, ch, 2, FEAT], f8, tag="src_chunk")
            nc.sync.dma_start(ct[:], src_drams[i][:])
            chunk_tiles.append(ct)

        s2c = _slot_to_chunk(chunk_sizes)

        for t in range(NTILES):
            agg = ps_agg.tile([P, FEAT], f32)
            kst = caps[t] + k_ovf[t]
            for k in range(kst):
                s = base[t] + k
                ci, cl = s2c[s]
                ct = chunk_tiles[ci]
                if k < caps[t]:
                    lhsT = identw_sb[:]
                else:
                    oh = ohp.tile([P, 2, P], f8)
                    nc.vector.tensor_scalar(
                        oh[:, 0, :], iota_sb[:], dstrel_sb[:, 2 * s : 2 * s + 1], None, eq
                    )
                    nc.vector.tensor_scalar(
                        oh[:, 1, :], iota_sb[:], dstrel_sb[:, 2 * s + 1 : 2 * s + 2], None, eq
                    )
                    lhsT = oh[:]
                nc.tensor.matmul(
                    agg[:],
                    lhsT,
                    ct[:, cl],
                    start=(k == 0),
                    stop=(k == kst - 1),
                    perf_mode=DR,
                )
            mean = meanp.tile([P, FEAT], f16)
            nc.vector.tensor_scalar(
                mean[:], agg[:], recip_sb[:, t : t + 1], None, mult
            )
            tp = ps_t.tile([P, 2, P], f16)
            nc.tensor.transpose(tp[:, 0, :], mean[:, 0:P], identt_sb[:])
            nc.tensor.transpose(tp[:, 1, :], mean[:, P : 2 * P], identt_sb[:])
            mt = mtp.tile([P, 2, P], f16)
            nc.vector.tensor_copy(mt[:], tp[:])
            op_ = ps_out.tile([P, FEAT], f32)
            nc.tensor.matmul(op_[:], mt[:, 0, :], wt_sb[:, 0, :], start=True, stop=False)
            nc.tensor.matmul(op_[:], mt[:, 1, :], wt_sb[:, 1, :], start=False, stop=True)
            ob = outp.tile([P, FEAT], f16)
            nc.vector.tensor_tensor(ob[:], op_[:], bias_sb[:], op=add)
            nc.scalar.dma_start(out_d[t], ob[:])

    nc.compile()
    if dedup:
        _postprocess_module(nc)
    return nc


def _postprocess_module(nc):
    """Two post-compile rewrites of the module JSON:

    1. Remove back-to-back redundant Ldweights on the PE stream (same weights
       AP, no new semaphore obligations): identity-slot chains reload the
       same stationary operand; Matmult keeps the last loaded weights.
    2. Hoist the leading wait-free DMA triggers (first src chunks + consts)
       out of the Tile body into `main` ahead of the all-engine init barrier
       so the first bytes stream during engine init."""
    import orjson
    from concourse import mybir

    raw = nc.to_json()
    removed = 0
    for fn in raw["functions"]:
        for blk in fn["blocks"]:
            out = []
            last_sig = None
            enforced = {}  # sem id -> max wait value already enforced on PE
            for inst in blk["instructions"]:
                if inst.get("engine") == "PE":
                    sync = inst.get("sync_info") or {}
                    waits = sync.get("on_wait") or []
                    if inst.get("opcode") == "Ldweights":
                        ups = sync.get("on_update") or []
                        sig = orjson.dumps(
                            {
                                k: v
                                for k, v in inst.items()
                                if k not in ("name", "debug", "sync_info")
                            },
                            option=orjson.OPT_SORT_KEYS,
                        )
                        if (
                            sig == last_sig
                            and not ups
                            and all(
                                w.get("sync_type") == "semaphore"
                                and isinstance(w.get("wait_value"), int)
                                and enforced.get(w["id"], -1) >= w["wait_value"]
                                for w in waits
                            )
                        ):
                            removed += 1
                            continue
                        last_sig = sig
                    for w in waits:
                        if w.get("sync_type") == "semaphore" and isinstance(
                            w.get("wait_value"), int
                        ):
                            enforced[w["id"]] = max(
                                enforced.get(w["id"], -1), w["wait_value"]
                            )
                out.append(inst)
            blk["instructions"] = out

    for fn in raw["functions"]:
        blocks = {b["name"]: b for b in fn["blocks"]}
        main = blocks.get("main")
        body = None
        for b in fn["blocks"]:
            if b["name"] != "main" and len(b["instructions"]) > 100:
                body = b
        if main is None or body is None:
            continue
        hoist = []
        kept = []
        for idx, inst in enumerate(body["instructions"]):
            if len(hoist) >= 10 or idx > 60:
                kept.extend(body["instructions"][idx:])
                break
            sync = inst.get("sync_info") or {}
            if inst.get("opcode") == "DMACopy" and not (sync.get("on_wait") or []):
                hoist.append(inst)
            else:
                kept.append(inst)
        if not hoist:
            continue
        body["instructions"] = kept
        mi = main["instructions"]
        pos = next(
            (i for i, x in enumerate(mi) if x.get("opcode") == "Drain"), len(mi)
        )
        main["instructions"] = mi[:pos] + hoist + mi[pos:]

    nc.m = mybir.parse_bytes(orjson.dumps(raw))
    return removed


def _prepare(inputs, dedup=True):
    """CPU-side sharding: returns (nc, in_maps) ready for SPMD dispatch."""
    src = np.asarray(inputs["source_node_representation_with_coefficient"])
    edge_index = np.asarray(inputs["edge_index"])
    W = np.asarray(inputs["W"], dtype=np.float32)
    b = np.asarray(inputs["b"], dtype=np.float32)
    assert src.shape == (N_EDGES, FEAT) and edge_index.shape == (2, N_EDGES)

    dst = edge_index[1].astype(np.int64)
    perm, dst_sorted, counts, tile_ranges, caps, k_ovf, base, nslot = _plan(dst)

    q = _quantize_ef(src, perm, dst_sorted, counts)  # [E+1, F] fp8, sorted order

    chunk_sizes = _chunk_schedule(nslot)
    nc = _build_program(caps, k_ovf, base, chunk_sizes, nslot, dedup=dedup)

    wt_packed = np.ascontiguousarray(
        W.T.reshape(2, P, FEAT).transpose(1, 0, 2)
    ).astype(np.float16)
    bias_tile = np.ascontiguousarray(np.broadcast_to(b, (P, FEAT))).astype(np.float16)
    iota_tile = np.ascontiguousarray(
        np.broadcast_to(np.arange(P, dtype=np.float16), (P, P))
    )
    identt_tile = np.eye(P, dtype=np.float32).astype(np.float16)
    identw_tile = np.zeros((P, 2, P), dtype=FP8)
    for j in range(2):
        identw_tile[np.arange(P), j, np.arange(P)] = 1.0

    # recip[p, t] = 1 / (max(count,1) * SCALE) for node t*128+p of this core
    pad = N_EDGES  # index of the all-zeros pad row in q

    in_maps = []
    for c in range(NCORES):
        pos = np.full((nslot, P, 2), pad, dtype=np.int64)  # sorted-order edge idx
        rel = np.zeros((nslot, P, 2), dtype=np.int64)
        for t in range(NTILES):
            lo, hi, n0, n1 = tile_ranges[c][t]
            n = hi - lo
            rows = n1 - n0
            b0 = base[t]
            C = caps[t]
            if n == 0:
                continue
            d_rel = dst_sorted[lo:hi] - n0  # sorted, in [0, rows)
            starts = np.searchsorted(d_rel, np.arange(rows))
            cnt_p = np.diff(np.append(starts, n))
            # identity slots: slot k half j, partition p <- edge 2k+j of node p
            kk = (2 * np.arange(C)[:, None, None] + np.arange(2)[None, None, :])
            valid = kk < cnt_p[None, :, None]  # [C, rows, 2]
            idx = np.minimum(starts[None, :, None] + kk, n - 1)
            pos[b0 : b0 + C, :rows] = np.where(valid, lo + idx, pad)
            # overflow edges: rank >= 2C within their node, packed densely
            rank = np.arange(n) - starts[d_rel]
            om = rank >= 2 * C
            novf = int(om.sum())
            if novf:
                ob0 = b0 + C
                tend = b0 + C + k_ovf[t]
                flat_pos = pos[ob0:tend].reshape(-1)
                flat_rel = rel[ob0:tend].reshape(-1)
                flat_pos[:novf] = lo + np.nonzero(om)[0]
                flat_rel[:novf] = d_rel[om]

        srcg = q[pos.reshape(-1)]  # [(nslot*P*2), F] fp8

        node0 = c * NPC
        cnt_core = np.zeros(NTILES * P, dtype=np.float64)
        ncv = min(NPC, N_NODES - node0)
        cnt_core[:ncv] = counts[node0 : node0 + ncv]
        recip = (1.0 / (np.maximum(cnt_core, 1.0) * SCALE)).astype(np.float32)
        recip_tile = np.ascontiguousarray(recip.reshape(NTILES, P).T)

        m = {
            "dstrel": np.ascontiguousarray(
                rel.transpose(1, 0, 2).reshape(P, 2 * nslot).astype(np.float32)
            ),
            "wt": wt_packed,
            "bias": bias_tile,
            "iota": iota_tile,
            "identw": identw_tile,
            "identt": identt_tile,
            "recip": recip_tile,
        }
        s0 = 0
        for i, ch in enumerate(chunk_sizes):
            blk = srcg[s0 * P * 2 : (s0 + ch) * P * 2].reshape(ch, P, 2, FEAT)
            m[f"src{i}"] = np.ascontiguousarray(blk.transpose(1, 0, 2, 3))
            s0 += ch
        in_maps.append(m)

    return nc, in_maps


def _gather_output(results):
    blocks = []
    for c in range(NCORES):
        o = np.asarray(results[c]["out"], dtype=np.float32)  # [NTILES, P, FEAT]
        o = o.reshape(NTILES * P, FEAT)[:NPC]
        blocks.append(o)
    return np.concatenate(blocks, axis=0)[:N_NODES]


def run(inputs, trace=False, **spmd_kwargs):
    from concourse.bass_utils import run_bass_kernel_spmd

    nc, in_maps = _prepare(inputs)
    res = run_bass_kernel_spmd(
        nc, in_maps, core_ids=list(range(NCORES)), trace=trace, **spmd_kwargs
    )
    return _gather_output(res.results), res


def kernel(**inputs) -> np.ndarray:
    out, _ = run(inputs, trace=False)
    return out
